# revision 32
# baseline (speedup 1.0000x reference)
"""MoE BERT head (soft routing) on 8 Trainium2 NeuronCores — v2.

Reference computation (B=4096, H=1024, E=32, O=1024):
    gate = softmax(X @ gate_W + gate_b, axis=experts)            [B, E]
    h_e  = relu(LN(X @ W1[e] + b1[e]) * ln_g[e] + ln_b[e])       [B, H] per expert
    out  = sum_e gate[:, e] * (h_e @ W2[e] + b2[e])              [B, O]

v2 strategy (expert-parallel, 4 experts/core, all-bf16 GEMMs):
  - gate softmax computed on HOST (0.27 GFLOP) and shipped as per-token
    scale factors; the per-expert output bias enters as gate @ b2 on host.
  - per chunk of 1024 tokens x 4 experts: GEMM1 (bf16) accumulates in PSUM,
    LayerNorm stats via bn_stats on PSUM (DVE), then the LN affine + gate
    fold + relu + bf16 cast all fused into the single PSUM->SBUF copy on the
    scalar engine: out = Relu(h*S + Bi), S = rstd*gate, Bi = -mu*S.
  - [token, feature] -> [feature, token] layout change via hardware DMA
    transpose (XBAR, 2-byte dtype) instead of PE transposes — keeps the
    tensor engine free for matmuls.
  - GEMM2 single pass: K-concat all 4 experts (32 k-tiles) into one PSUM
    accumulation group; per-core partial [O, B] fp32 summed on host.
All heavy math is bf16 x bf16 -> fp32 PSUM (max rel err vs fp32 reference
~2e-3, tolerance 2e-2).
"""

import os
import sys
from contextlib import ExitStack

for _p in ("/opt/trn_rl_repo", "/root/.axon_site/_ro/trn_rl_repo"):
    if os.path.isdir(_p) and _p not in sys.path:
        sys.path.insert(0, _p)

import numpy as np
import ml_dtypes

import concourse.bass as bass
import concourse.mybir as mybir
import concourse.tile as tile
from concourse import bacc
from concourse.bass_utils import run_bass_kernel_spmd

B, H, E, O = 4096, 1024, 32, 1024
LN_EPS = 1e-5
N_CORES = 8
E_PER_CORE = E // N_CORES            # 4
B_CHUNK = 1024
N_CHUNKS = B // B_CHUNK              # 4
P = 128
KT = H // P                          # 8 k-tiles over the hidden dim
BT = B_CHUNK // P                    # 8 token tiles per chunk
OT = O // P                          # 8 output tiles
F32 = mybir.dt.float32
BF16 = mybir.dt.bfloat16

Relu = mybir.ActivationFunctionType.Relu
Sqrt = mybir.ActivationFunctionType.Sqrt
Copy = mybir.ActivationFunctionType.Copy
Alu = mybir.AluOpType

_CACHE = {}
_LAST_IN_MAPS = None


def _build_program(use_b1, use_lng, use_lnb):
    nc = bacc.Bacc("TRN2", target_bir_lowering=False, debug=False,
                   num_devices=N_CORES)

    xt_d = nc.dram_tensor("xt", [P, KT, B], BF16, kind="ExternalInput")
    w1_d = nc.dram_tensor("w1", [E_PER_CORE, P, KT, 2, 512], BF16,
                          kind="ExternalInput")
    w2_d = nc.dram_tensor("w2t", [2, OT, P, 2 * KT, P], BF16,
                          kind="ExternalInput")
    g2_d = nc.dram_tensor("g2", [P, E_PER_CORE, N_CHUNKS * BT], F32,
                          kind="ExternalInput")
    b1_d = (nc.dram_tensor("b1", [E_PER_CORE, H], F32, kind="ExternalInput")
            if use_b1 else None)
    lng_d = (nc.dram_tensor("lng", [E_PER_CORE, H], F32, kind="ExternalInput")
             if use_lng else None)
    lnb_d = (nc.dram_tensor("lnb", [E_PER_CORE, H], F32, kind="ExternalInput")
             if use_lnb else None)
    out_d = nc.dram_tensor("outp", [2, O, B], F32, kind="ExternalOutput")

    general = use_b1 or use_lng or use_lnb

    with tile.TileContext(nc) as tc, ExitStack() as ctx:
        pool = lambda name, bufs, **kw: ctx.enter_context(
            tc.tile_pool(name=name, bufs=bufs, **kw))
        singles = pool("singles", 1)
        xtp = pool("xtp", 2)
        w1p = pool("w1p", 2)
        w2p = pool("w2p", 4)
        hpp = pool("hpp", 2)          # h' tiles, 8 tags (one per bt)
        hstp = pool("hstp", 2)        # hsT per expert PAIR, double-buffered
        osbp = pool("osbp", 3)
        smallp = pool("smallp", 24)
        hps = pool("hps", 3, space="PSUM")
        ops = pool("ops", 2, space="PSUM")
        bcp = pool("bcp", 1) if general else None
        gtp = pool("gtp", 2) if general else None

        eps_t = singles.tile([P, 1], F32)
        nc.vector.memset(eps_t, LN_EPS)
        g2_sb = singles.tile([P, E_PER_CORE, N_CHUNKS * BT], F32)
        nc.sync.dma_start(out=g2_sb[:], in_=g2_d[:])


        # pending GEMM2 work for the last completed expert pair:
        # [hsT_tile, p_i, c0] — consumed one (ot, bs) chain per subsequent
        # GEMM1 token-tile so the PE stream stays uniform and the LN
        # post-processing latency is fully hidden.
        pending = [None]
        w2_cur = [None]

        def emit_g2(idx):
            hsT_p, p_i, pc0 = pending[0]
            ot, bs = idx // 2, idx % 2
            if bs == 0:
                w2t = w2p.tile([P, 2 * KT, P], BF16, tag="w2", name="w2t")
                nc.sync.dma_start(out=w2t[:], in_=w2_d[p_i, ot])
                w2_cur[0] = w2t
            op = ops.tile([P, 512], F32, tag="op")
            for ke in range(2 * KT):
                nc.tensor.matmul(
                    op[:], w2_cur[0][:, ke, :],
                    hsT_p[:, ke, bs * 512:(bs + 1) * 512],
                    start=(ke == 0), stop=(ke == 2 * KT - 1))
            osb = osbp.tile([P, 512], F32, tag="osb")
            nc.vector.tensor_copy(osb[:], op[:])
            nc.sync.dma_start(
                out=out_d[p_i, ot * P:(ot + 1) * P,
                          pc0 + bs * 512:pc0 + (bs + 1) * 512],
                in_=osb[:])

        for c in range(N_CHUNKS):
            c0 = c * B_CHUNK
            xt = xtp.tile([P, KT, B_CHUNK], BF16, tag="xt")
            nc.sync.dma_start(out=xt[:], in_=xt_d[:, :, c0:c0 + B_CHUNK])

            hsT = None
            for e in range(E_PER_CORE):
                e01 = e % 2
                if e01 == 0:
                    hsT = hstp.tile([P, 2 * KT, B_CHUNK], BF16, tag="hsT")
                w1t = w1p.tile([P, KT, 2, 512], BF16, tag="w1")
                nc.sync.dma_start(out=w1t[:], in_=w1_d[e])
                b1_bc = lng_bc = lnb_bc = None
                if use_b1:
                    b1_bc = bcp.tile([P, H], F32, tag="b1bc")
                    nc.gpsimd.dma_start(out=b1_bc[:],
                                        in_=b1_d[e].partition_broadcast(P))
                if use_lng:
                    lng_bc = bcp.tile([P, H], F32, tag="lngbc")
                    nc.gpsimd.dma_start(out=lng_bc[:],
                                        in_=lng_d[e].partition_broadcast(P))
                if use_lnb:
                    lnb_bc = bcp.tile([P, H], F32, tag="lnbbc")
                    nc.gpsimd.dma_start(out=lnb_bc[:],
                                        in_=lnb_d[e].partition_broadcast(P))

                for bt in range(BT):
                    ps2 = [hps.tile([P, 512], F32, tag=f"ps{d}", name=f"ps{d}")
                           for d in range(2)]
                    for dch in range(2):
                        for k in range(KT):
                            nc.tensor.matmul(
                                ps2[dch][:],
                                xt[:, k, bt * P:(bt + 1) * P],
                                w1t[:, k, dch, :],
                                start=(k == 0), stop=(k == KT - 1))

                    # LayerNorm stats over the feature dim
                    stats = smallp.tile([P, 2, 6], F32, tag="stats")
                    hg = None
                    if use_b1:
                        hg = gtp.tile([P, H], F32, tag="hg")
                        for dch in range(2):
                            nc.vector.tensor_add(
                                hg[:, dch * 512:(dch + 1) * 512],
                                ps2[dch][:], b1_bc[:, dch * 512:(dch + 1) * 512])
                            nc.vector.bn_stats(stats[:, dch, :],
                                               hg[:, dch * 512:(dch + 1) * 512])
                    else:
                        for dch in range(2):
                            nc.vector.bn_stats(stats[:, dch, :], ps2[dch][:])
                    mv = smallp.tile([P, 2], F32, tag="mv")
                    nc.vector.bn_aggr(mv[:], stats[:])
                    rg = smallp.tile([P, 1], F32, tag="rg")
                    nc.scalar.activation(rg[:], mv[:, 1:2], Sqrt, bias=eps_t[:])
                    nc.vector.reciprocal(rg[:], rg[:])
                    gate_ap = g2_sb[:, e, c * BT + bt:c * BT + bt + 1]
                    S = smallp.tile([P, 1], F32, tag="S")
                    nc.vector.tensor_mul(S[:], rg[:], gate_ap)
                    Bi = smallp.tile([P, 1], F32, tag="Bi")
                    nc.vector.tensor_mul(Bi[:], mv[:, 0:1], S[:])
                    nc.vector.tensor_scalar_mul(Bi[:], Bi[:], -1.0)

                    hpt = hpp.tile([P, B_CHUNK], BF16, tag=f"hp{bt}",
                                   name=f"hp{bt}")
                    if not general:
                        # fused: h' = Relu(h * S + Bi), cast to bf16
                        for dch in range(2):
                            nc.scalar.activation(
                                hpt[:, dch * 512:(dch + 1) * 512],
                                ps2[dch][:], Relu, bias=Bi[:], scale=S[:])
                    else:
                        for dch in range(2):
                            dst = hpt[:, dch * 512:(dch + 1) * 512]
                            src = (hg[:, dch * 512:(dch + 1) * 512]
                                   if use_b1 else ps2[dch][:])
                            # (h - mu) * (rstd * gate)
                            nc.vector.tensor_scalar(
                                dst, src, mv[:, 0:1], S[:],
                                op0=Alu.subtract, op1=Alu.mult)
                        if use_lng:
                            nc.vector.tensor_mul(hpt[:], hpt[:], lng_bc[:])
                        if use_lnb:
                            nc.vector.scalar_tensor_tensor(
                                hpt[:], lnb_bc[:], gate_ap, hpt[:],
                                op0=Alu.mult, op1=Alu.add)
                        nc.vector.tensor_scalar_max(hpt[:], hpt[:], 0.0)

                    # HW DMA transpose into the pair buffer
                    nc.sync.dma_start_transpose(
                        out=hsT[:, e01 * KT:(e01 + 1) * KT,
                                bt * P:(bt + 1) * P],
                        in_=hpt[:])

                    # interleave one GEMM2 chain of the previous pair
                    if pending[0] is not None:
                        idx = (e01 * BT + bt)
                        emit_g2(idx)

                if e01 == 1:
                    # this pair's GEMM2 runs interleaved with the NEXT two
                    # experts' GEMM1 token-tiles (16 chains over 16 tiles)
                    pending[0] = [hsT, e // 2, c0]

        # flush the final pair's GEMM2
        for idx in range(2 * BT):
            emit_g2(idx)

    nc.compile()
    return nc


def kernel(pooled_output, gate_W, gate_b, W1, b1, ln_g, ln_b, W2, b2):
    X = np.asarray(pooled_output, dtype=np.float32)
    gate_W = np.asarray(gate_W, dtype=np.float32)
    gate_b = np.asarray(gate_b, dtype=np.float32)
    W1 = np.asarray(W1, dtype=np.float32)
    b1 = np.asarray(b1, dtype=np.float32)
    ln_g = np.asarray(ln_g, dtype=np.float32)
    ln_b = np.asarray(ln_b, dtype=np.float32)
    W2 = np.asarray(W2, dtype=np.float32)
    b2 = np.asarray(b2, dtype=np.float32)

    use_b1 = bool(np.any(b1 != 0.0))
    use_lng = bool(np.any(ln_g != 1.0))
    use_lnb = bool(np.any(ln_b != 0.0))

    key = (use_b1, use_lng, use_lnb)
    if key not in _CACHE:
        _CACHE[key] = _build_program(*key)
    nc = _CACHE[key]

    # host gate softmax (exact)
    gate = X @ gate_W + gate_b[None, :]
    gate -= gate.max(axis=1, keepdims=True)
    np.exp(gate, out=gate)
    gate /= gate.sum(axis=1, keepdims=True)          # [B, E] fp32

    # X^T tiled [P, KT, B] bf16
    xt_h = np.ascontiguousarray(
        X.T.reshape(KT, P, B).transpose(1, 0, 2)).astype(ml_dtypes.bfloat16)

    in_maps = []
    for c in range(N_CORES):
        own = list(range(E_PER_CORE * c, E_PER_CORE * (c + 1)))
        w1_c = W1[own].reshape(E_PER_CORE, KT, P, 2, 512)
        w1_c = np.ascontiguousarray(w1_c.transpose(0, 2, 1, 3, 4)).astype(
            ml_dtypes.bfloat16)
        w2_c = W2[own].reshape(2, 2, KT, P, OT, P)
        w2_c = np.ascontiguousarray(w2_c.transpose(0, 4, 3, 1, 2, 5)).astype(
            ml_dtypes.bfloat16)
        w2_c = w2_c.reshape(2, OT, P, 2 * KT, P)
        g2 = np.ascontiguousarray(
            gate[:, own].reshape(N_CHUNKS * BT, P, E_PER_CORE)
            .transpose(1, 2, 0))                      # [P, E_loc, 32]
        m = {"xt": xt_h, "w1": w1_c, "w2t": w2_c, "g2": g2}
        if use_b1:
            m["b1"] = np.ascontiguousarray(b1[own])
        if use_lng:
            m["lng"] = np.ascontiguousarray(ln_g[own])
        if use_lnb:
            m["lnb"] = np.ascontiguousarray(ln_b[own])
        in_maps.append(m)

    global _LAST_IN_MAPS
    _LAST_IN_MAPS = in_maps
    res = run_bass_kernel_spmd(nc, in_maps, core_ids=list(range(N_CORES)))

    acc = np.zeros((O, B), dtype=np.float32)
    for c in range(N_CORES):
        part = res.results[c]["outp"]
        acc += part[0]
        acc += part[1]
    out = np.ascontiguousarray(acc.T)
    if np.any(b2 != 0.0):
        out += gate @ b2
    return np.ascontiguousarray(out, dtype=np.float32)


# revision 33
# speedup vs baseline: 1.3036x; 1.3036x over previous
"""MoE BERT head (soft routing) on 8 Trainium2 NeuronCores — v2.

Reference computation (B=4096, H=1024, E=32, O=1024):
    gate = softmax(X @ gate_W + gate_b, axis=experts)            [B, E]
    h_e  = relu(LN(X @ W1[e] + b1[e]) * ln_g[e] + ln_b[e])       [B, H] per expert
    out  = sum_e gate[:, e] * (h_e @ W2[e] + b2[e])              [B, O]

v2 strategy (expert-parallel, 4 experts/core, all-bf16 GEMMs):
  - gate softmax computed on HOST (0.27 GFLOP) and shipped as per-token
    scale factors; the per-expert output bias enters as gate @ b2 on host.
  - per chunk of 1024 tokens x 4 experts: GEMM1 (bf16) accumulates in PSUM,
    LayerNorm stats via bn_stats on PSUM (DVE), then the LN affine + gate
    fold + relu + bf16 cast all fused into the single PSUM->SBUF copy on the
    scalar engine: out = Relu(h*S + Bi), S = rstd*gate, Bi = -mu*S.
  - [token, feature] -> [feature, token] layout change via hardware DMA
    transpose (XBAR, 2-byte dtype) instead of PE transposes — keeps the
    tensor engine free for matmuls.
  - GEMM2 single pass: K-concat all 4 experts (32 k-tiles) into one PSUM
    accumulation group; per-core partial [O, B] fp32 summed on host.
All heavy math is bf16 x bf16 -> fp32 PSUM (max rel err vs fp32 reference
~2e-3, tolerance 2e-2).
"""

import os
import sys
from contextlib import ExitStack

for _p in ("/opt/trn_rl_repo", "/root/.axon_site/_ro/trn_rl_repo"):
    if os.path.isdir(_p) and _p not in sys.path:
        sys.path.insert(0, _p)

import numpy as np
import ml_dtypes

import concourse.bass as bass
import concourse.mybir as mybir
import concourse.tile as tile
from concourse import bacc
from concourse.bass_utils import run_bass_kernel_spmd

B, H, E, O = 4096, 1024, 32, 1024
LN_EPS = 1e-5
N_CORES = 8
E_PER_CORE = E // N_CORES            # 4
B_CHUNK = 1024
N_CHUNKS = B // B_CHUNK              # 4
P = 128
KT = H // P                          # 8 k-tiles over the hidden dim
BT = B_CHUNK // P                    # 8 token tiles per chunk
OT = O // P                          # 8 output tiles
F32 = mybir.dt.float32
BF16 = mybir.dt.bfloat16

Relu = mybir.ActivationFunctionType.Relu
Sqrt = mybir.ActivationFunctionType.Sqrt
Copy = mybir.ActivationFunctionType.Copy
Alu = mybir.AluOpType

_CACHE = {}
_LAST_IN_MAPS = None


def _build_program(use_b1, use_lng, use_lnb):
    nc = bacc.Bacc("TRN2", target_bir_lowering=False, debug=False,
                   num_devices=N_CORES)

    xt_d = nc.dram_tensor("xt", [P, KT, B], BF16, kind="ExternalInput")
    w1_d = nc.dram_tensor("w1", [E_PER_CORE, 2, P, KT, 512], BF16,
                          kind="ExternalInput")
    w2_d = nc.dram_tensor("w2t", [2, OT, P, 2 * KT, P], BF16,
                          kind="ExternalInput")
    g2_d = nc.dram_tensor("g2", [P, E_PER_CORE, N_CHUNKS * BT], F32,
                          kind="ExternalInput")
    b1_d = (nc.dram_tensor("b1", [E_PER_CORE, H], F32, kind="ExternalInput")
            if use_b1 else None)
    lng_d = (nc.dram_tensor("lng", [E_PER_CORE, H], F32, kind="ExternalInput")
             if use_lng else None)
    lnb_d = (nc.dram_tensor("lnb", [E_PER_CORE, H], F32, kind="ExternalInput")
             if use_lnb else None)
    out_d = nc.dram_tensor("outp", [2, O, B], F32, kind="ExternalOutput")

    general = use_b1 or use_lng or use_lnb

    with tile.TileContext(nc) as tc, ExitStack() as ctx:
        pool = lambda name, bufs, **kw: ctx.enter_context(
            tc.tile_pool(name=name, bufs=bufs, **kw))
        singles = pool("singles", 1)
        xtp = pool("xtp", 2)
        w1p = pool("w1p", 4)
        w2p = pool("w2p", 3)
        hpp = pool("hpp", 2)          # h' tiles, 8 tags (one per bt)
        hstp = pool("hstp", 2)        # hsT per expert PAIR, double-buffered
        osbp = pool("osbp", 3)
        smallp = pool("smallp", 24)
        hps = pool("hps", 3, space="PSUM")
        ops = pool("ops", 2, space="PSUM")
        bcp = pool("bcp", 1) if general else None
        gtp = pool("gtp", 2) if general else None

        eps_t = singles.tile([P, 1], F32)
        nc.vector.memset(eps_t, LN_EPS)
        g2_sb = singles.tile([P, E_PER_CORE, N_CHUNKS * BT], F32)
        nc.sync.dma_start(out=g2_sb[:], in_=g2_d[:])


        # pending GEMM2 work for the last completed expert pair:
        # [hsT_tile, p_i, c0] — consumed one (ot, bs) chain per subsequent
        # GEMM1 token-tile so the PE stream stays uniform and the LN
        # post-processing latency is fully hidden.
        pending = [None]
        w2_cur = [None]

        def emit_g2(idx):
            hsT_p, p_i, pc0 = pending[0]
            ot, bs = idx // 2, idx % 2
            if bs == 0:
                w2t = w2p.tile([P, 2 * KT, P], BF16, tag="w2", name="w2t")
                nc.sync.dma_start(out=w2t[:], in_=w2_d[p_i, ot])
                w2_cur[0] = w2t
            op = ops.tile([P, 512], F32, tag="op")
            for ke in range(2 * KT):
                nc.tensor.matmul(
                    op[:], w2_cur[0][:, ke, :],
                    hsT_p[:, ke, bs * 512:(bs + 1) * 512],
                    start=(ke == 0), stop=(ke == 2 * KT - 1))
            osb = osbp.tile([P, 512], F32, tag="osb")
            nc.vector.tensor_copy(osb[:], op[:])
            nc.sync.dma_start(
                out=out_d[p_i, ot * P:(ot + 1) * P,
                          pc0 + bs * 512:pc0 + (bs + 1) * 512],
                in_=osb[:])

        for c in range(N_CHUNKS):
            c0 = c * B_CHUNK
            xt = xtp.tile([P, KT, B_CHUNK], BF16, tag="xt")
            nc.sync.dma_start(out=xt[:], in_=xt_d[:, :, c0:c0 + B_CHUNK])

            hsT = None
            for e in range(E_PER_CORE):
                e01 = e % 2
                if e01 == 0:
                    hsT = hstp.tile([P, 2 * KT, B_CHUNK], BF16, tag="hsT")
                w1t = []
                for dch in range(2):
                    t = w1p.tile([P, KT, 512], BF16, tag="w1", name=f"w1_{dch}")
                    nc.sync.dma_start(out=t[:], in_=w1_d[e, dch])
                    w1t.append(t)
                b1_bc = lng_bc = lnb_bc = None
                if use_b1:
                    b1_bc = bcp.tile([P, H], F32, tag="b1bc")
                    nc.gpsimd.dma_start(out=b1_bc[:],
                                        in_=b1_d[e].partition_broadcast(P))
                if use_lng:
                    lng_bc = bcp.tile([P, H], F32, tag="lngbc")
                    nc.gpsimd.dma_start(out=lng_bc[:],
                                        in_=lng_d[e].partition_broadcast(P))
                if use_lnb:
                    lnb_bc = bcp.tile([P, H], F32, tag="lnbbc")
                    nc.gpsimd.dma_start(out=lnb_bc[:],
                                        in_=lnb_d[e].partition_broadcast(P))

                for bt in range(BT):
                    ps2 = [hps.tile([P, 512], F32, tag=f"ps{d}", name=f"ps{d}")
                           for d in range(2)]
                    for dch in range(2):
                        for k in range(KT):
                            nc.tensor.matmul(
                                ps2[dch][:],
                                xt[:, k, bt * P:(bt + 1) * P],
                                w1t[dch][:, k, :],
                                start=(k == 0), stop=(k == KT - 1))

                    # LayerNorm stats over the feature dim
                    stats = smallp.tile([P, 2, 6], F32, tag="stats")
                    hg = None
                    if use_b1:
                        hg = gtp.tile([P, H], F32, tag="hg")
                        for dch in range(2):
                            nc.vector.tensor_add(
                                hg[:, dch * 512:(dch + 1) * 512],
                                ps2[dch][:], b1_bc[:, dch * 512:(dch + 1) * 512])
                            nc.vector.bn_stats(stats[:, dch, :],
                                               hg[:, dch * 512:(dch + 1) * 512])
                    else:
                        for dch in range(2):
                            nc.vector.bn_stats(stats[:, dch, :], ps2[dch][:])
                    mv = smallp.tile([P, 2], F32, tag="mv")
                    nc.vector.bn_aggr(mv[:], stats[:])
                    rg = smallp.tile([P, 1], F32, tag="rg")
                    nc.scalar.activation(rg[:], mv[:, 1:2], Sqrt, bias=eps_t[:])
                    nc.vector.reciprocal(rg[:], rg[:])
                    gate_ap = g2_sb[:, e, c * BT + bt:c * BT + bt + 1]
                    S = smallp.tile([P, 1], F32, tag="S")
                    nc.vector.tensor_mul(S[:], rg[:], gate_ap)
                    Bi = smallp.tile([P, 1], F32, tag="Bi")
                    nc.vector.tensor_mul(Bi[:], mv[:, 0:1], S[:])
                    nc.vector.tensor_scalar_mul(Bi[:], Bi[:], -1.0)

                    hpt = hpp.tile([P, B_CHUNK], BF16, tag=f"hp{bt}",
                                   name=f"hp{bt}")
                    if not general:
                        # fused: h' = Relu(h * S + Bi), cast to bf16
                        for dch in range(2):
                            nc.scalar.activation(
                                hpt[:, dch * 512:(dch + 1) * 512],
                                ps2[dch][:], Relu, bias=Bi[:], scale=S[:])
                    else:
                        for dch in range(2):
                            dst = hpt[:, dch * 512:(dch + 1) * 512]
                            src = (hg[:, dch * 512:(dch + 1) * 512]
                                   if use_b1 else ps2[dch][:])
                            # (h - mu) * (rstd * gate)
                            nc.vector.tensor_scalar(
                                dst, src, mv[:, 0:1], S[:],
                                op0=Alu.subtract, op1=Alu.mult)
                        if use_lng:
                            nc.vector.tensor_mul(hpt[:], hpt[:], lng_bc[:])
                        if use_lnb:
                            nc.vector.scalar_tensor_tensor(
                                hpt[:], lnb_bc[:], gate_ap, hpt[:],
                                op0=Alu.mult, op1=Alu.add)
                        nc.vector.tensor_scalar_max(hpt[:], hpt[:], 0.0)

                    # HW DMA transpose into the pair buffer
                    nc.sync.dma_start_transpose(
                        out=hsT[:, e01 * KT:(e01 + 1) * KT,
                                bt * P:(bt + 1) * P],
                        in_=hpt[:])

                    # interleave one GEMM2 chain of the previous pair
                    if pending[0] is not None:
                        idx = (e01 * BT + bt)
                        emit_g2(idx)

                if e01 == 1:
                    # this pair's GEMM2 runs interleaved with the NEXT two
                    # experts' GEMM1 token-tiles (16 chains over 16 tiles)
                    pending[0] = [hsT, e // 2, c0]

        # flush the final pair's GEMM2
        for idx in range(2 * BT):
            emit_g2(idx)

    nc.compile()
    return nc


def kernel(pooled_output, gate_W, gate_b, W1, b1, ln_g, ln_b, W2, b2):
    X = np.asarray(pooled_output, dtype=np.float32)
    gate_W = np.asarray(gate_W, dtype=np.float32)
    gate_b = np.asarray(gate_b, dtype=np.float32)
    W1 = np.asarray(W1, dtype=np.float32)
    b1 = np.asarray(b1, dtype=np.float32)
    ln_g = np.asarray(ln_g, dtype=np.float32)
    ln_b = np.asarray(ln_b, dtype=np.float32)
    W2 = np.asarray(W2, dtype=np.float32)
    b2 = np.asarray(b2, dtype=np.float32)

    use_b1 = bool(np.any(b1 != 0.0))
    use_lng = bool(np.any(ln_g != 1.0))
    use_lnb = bool(np.any(ln_b != 0.0))

    key = (use_b1, use_lng, use_lnb)
    if key not in _CACHE:
        _CACHE[key] = _build_program(*key)
    nc = _CACHE[key]

    # host gate softmax (exact)
    gate = X @ gate_W + gate_b[None, :]
    gate -= gate.max(axis=1, keepdims=True)
    np.exp(gate, out=gate)
    gate /= gate.sum(axis=1, keepdims=True)          # [B, E] fp32

    # X^T tiled [P, KT, B] bf16
    xt_h = np.ascontiguousarray(
        X.T.reshape(KT, P, B).transpose(1, 0, 2)).astype(ml_dtypes.bfloat16)

    in_maps = []
    for c in range(N_CORES):
        own = list(range(E_PER_CORE * c, E_PER_CORE * (c + 1)))
        w1_c = W1[own].reshape(E_PER_CORE, KT, P, 2, 512)
        w1_c = np.ascontiguousarray(w1_c.transpose(0, 3, 2, 1, 4)).astype(
            ml_dtypes.bfloat16)
        w2_c = W2[own].reshape(2, 2, KT, P, OT, P)
        w2_c = np.ascontiguousarray(w2_c.transpose(0, 4, 3, 1, 2, 5)).astype(
            ml_dtypes.bfloat16)
        w2_c = w2_c.reshape(2, OT, P, 2 * KT, P)
        g2 = np.ascontiguousarray(
            gate[:, own].reshape(N_CHUNKS * BT, P, E_PER_CORE)
            .transpose(1, 2, 0))                      # [P, E_loc, 32]
        m = {"xt": xt_h, "w1": w1_c, "w2t": w2_c, "g2": g2}
        if use_b1:
            m["b1"] = np.ascontiguousarray(b1[own])
        if use_lng:
            m["lng"] = np.ascontiguousarray(ln_g[own])
        if use_lnb:
            m["lnb"] = np.ascontiguousarray(ln_b[own])
        in_maps.append(m)

    global _LAST_IN_MAPS
    _LAST_IN_MAPS = in_maps
    res = run_bass_kernel_spmd(nc, in_maps, core_ids=list(range(N_CORES)))

    acc = np.zeros((O, B), dtype=np.float32)
    for c in range(N_CORES):
        part = res.results[c]["outp"]
        acc += part[0]
        acc += part[1]
    out = np.ascontiguousarray(acc.T)
    if np.any(b2 != 0.0):
        out += gate @ b2
    return np.ascontiguousarray(out, dtype=np.float32)
